# revision 63
# baseline (speedup 1.0000x reference)
"""Trainium2 Bass kernel for two-stage pooled-query attention.

Problem (hardcoded):
    B=32, N=577, C=1024, H=16 heads, d=64, pooled queries 8x8 (3x3 mean over
    24x24 grid of non-cls tokens).
    qkv = X @ W_qkv.T ; pool Xq -> Qp ; s1 = softmax(Qp*s @ K^T) @ V ;
    s2 = softmax(Xq*s @ Qp^T) @ s1 ; out = s2 @ W_proj.T + b_proj

Strategy: pure data-parallel over batch across 8 NeuronCores (4 batches per
core, no collectives). All matmuls run bf16 with fp32 PSUM accumulation.
Layout is chosen so every contraction sits on SBUF partitions, and every
matmul uses a full K=128 contraction (partial-K matmuls misbehave on HW):
  - X is PE-transposed once into XT [k, n] (zero-padded to 640 tokens).
  - QKV GEMM emits Xq/Xk transposed [c, n] and V natural [n, c].
  - Pooling is a strided-AP reduce over XqT columns (exact 3x3 mean).
  - Per head-pair, pooled queries go into a block-diagonal [128, 128] lhsT
    so both heads' scores come from one K=128 matmul.
  - Stage-1 Qd is computed as a full [2q, 2dv] pair product; only the
    per-head diagonal blocks are kept (and softmax-normalized) on evict.
  - Attention output is produced transposed [c, n], which is exactly the
    lhsT layout the output projection needs; bias is pre-broadcast once
    and added during the output evict copy.
"""

import os
import sys

import numpy as np

sys.path.insert(0, "/opt/trn_rl_repo")

import ml_dtypes  # noqa: E402

import concourse.tile as tile  # noqa: E402
from concourse import bacc, mybir  # noqa: E402
from concourse.bass_utils import run_bass_kernel_spmd  # noqa: E402
from concourse.masks import make_identity  # noqa: E402

B, N, C = 32, 577, 1024
H, D = 16, 64
SCALE = D ** -0.5
N_CORES = 8
NB = B // N_CORES  # batches per core

BF16 = mybir.dt.bfloat16
F32 = mybir.dt.float32

# token chunks of 577 = 4*128 + 65
TOK = [(0, 128), (128, 128), (256, 128), (384, 128), (512, 65)]
# free-dim chunks of 577 for wide matmuls / psum banks
NF = [(0, 320), (320, 257)]
EXP = mybir.ActivationFunctionType.Exp


def build_program(nb: int = NB):
    nc = bacc.Bacc("TRN2", target_bir_lowering=False, debug=False)

    x_d = nc.dram_tensor("x", [nb, N, C], BF16, kind="ExternalInput")
    wqkvt_d = nc.dram_tensor("wqkvt", [C, 3 * C], BF16, kind="ExternalInput")
    wprojt_d = nc.dram_tensor("wprojt", [C, C], BF16, kind="ExternalInput")
    wbias_d = nc.dram_tensor("wbias", [1, C], BF16, kind="ExternalInput")
    out_d = nc.dram_tensor("out", [nb, N, C], F32, kind="ExternalOutput")

    with tile.TileContext(nc) as tc:
        const_pool = tc.alloc_tile_pool(name="const", bufs=1)
        w_pool = tc.alloc_tile_pool(name="w", bufs=1)
        sb = tc.alloc_tile_pool(name="sb", bufs=2)
        ps_big = tc.alloc_tile_pool(name="ps_big", bufs=5, space="PSUM")
        ps_small = tc.alloc_tile_pool(name="ps_small", bufs=3, space="PSUM")

        ident = const_pool.tile([128, 128], BF16, tag="ident")
        make_identity(nc, ident[:])
        ones = const_pool.tile([1, 128], BF16, tag="ones")
        nc.gpsimd.memset(ones[:], 1.0)

        # first batch's X goes out before the (much larger) weight DMAs so
        # the PE can start transposing immediately; weights stream behind.
        X0 = []
        for t, (toff, rows) in enumerate(TOK):
            xt = sb.tile([128, C], BF16, tag=f"x{t}", bufs=1)
            nc.sync.dma_start(xt[:rows, :], x_d[0, toff : toff + rows, :])
            X0.append(xt)

        # resident weights, streamed in 512-col chunks: q/k columns first so
        # the QKV gemm can start as soon as its first chunks land, V and
        # proj weights trail behind.
        wq = []
        for j in range(8):
            wqt = w_pool.tile([128, 3 * C], BF16, tag=f"wq{j}")
            wq.append(wqt)
        for blk in range(4):
            for j in range(8):
                cs = slice(512 * blk, 512 * (blk + 1))
                nc.sync.dma_start(wq[j][:, cs], wqkvt_d[128 * j : 128 * (j + 1), cs])
        for blk in range(4, 6):
            for j in range(8):
                cs = slice(512 * blk, 512 * (blk + 1))
                nc.sync.dma_start(wq[j][:, cs], wqkvt_d[128 * j : 128 * (j + 1), cs])
        wp = []
        for j in range(8):
            t = w_pool.tile([128, C], BF16, tag=f"wp{j}")
            nc.sync.dma_start(t[:], wprojt_d[128 * j : 128 * (j + 1), :])
            wp.append(t)
        wb = w_pool.tile([1, C], BF16, tag="wb")
        nc.sync.dma_start(wb[:], wbias_d[:])

        # bias broadcast [128, 1024]; built lazily (first use is phase 8)
        bias = const_pool.tile([128, C], BF16, tag="bias")
        bias_built = [False]

        def build_bias():
            if bias_built[0]:
                return
            bias_built[0] = True
            for half in range(2):
                cs = slice(512 * half, 512 * (half + 1))
                bps = ps_big.tile([128, 512], F32, tag="pbig")
                nc.tensor.matmul(
                    bps[:], ones[0:1, :], wb[0:1, cs], start=True, stop=True
                )
                nc.any.tensor_copy(bias[:, cs], bps[:])

        repeat = int(os.environ.get("KERNEL_REPEAT", "1"))
        for b in [bb for _ in range(repeat) for bb in range(nb)]:
            # ---- Phase 1: load X, transpose to XT [k, n], pad tail ----
            if b == 0 and X0 is not None:
                X, X0 = X0, None
            else:
                X = []
                for t, (toff, rows) in enumerate(TOK):
                    xt = sb.tile([128, C], BF16, tag=f"x{t}", bufs=1)
                    nc.sync.dma_start(xt[:rows, :], x_d[b, toff : toff + rows, :])
                    X.append(xt)
            XT = []
            for j in range(8):
                ks = slice(128 * j, 128 * (j + 1))
                pa = ps_big.tile([128, 512], BF16, tag="pbig")
                for t in range(4):
                    nc.tensor.transpose(
                        pa[:, 128 * t : 128 * (t + 1)], X[t][:, ks], ident[:]
                    )
                pb = ps_small.tile([128, 128], BF16, tag="psmall")
                nc.tensor.transpose(pb[:, 0:65], X[4][0:65, ks], ident[0:65, 0:65])
                xtj = sb.tile([128, 640], BF16, tag=f"xt{j}", bufs=2)
                nc.any.tensor_copy(xtj[:, 0:512], pa[:])
                nc.any.tensor_copy(xtj[:, 512:577], pb[:, 0:65])
                nc.any.memset(xtj[:, 577:640], 0.0)
                XT.append(xtj)

            # ---- Phase 2: QKV gemm, q/k parts transposed: qkT [c, n] ----
            qkT = []
            for cc in range(16):
                qt = sb.tile([128, 640], BF16, tag=f"qkt{cc}", bufs=1)
                for ci, (n0, nw) in enumerate(NF):
                    ps = ps_big.tile([128, nw], F32, tag="pbig")
                    for j in range(8):
                        nc.tensor.matmul(
                            ps[:],
                            wq[j][:, 128 * cc : 128 * (cc + 1)],
                            XT[j][:, n0 : n0 + nw],
                            start=(j == 0),
                            stop=(j == 7),
                        )
                    if (cc + ci) % 2 == 0:
                        nc.vector.tensor_copy(qt[:, n0 : n0 + nw], ps[:])
                    else:
                        nc.scalar.copy(qt[:, n0 : n0 + nw], ps[:])
                nc.any.memset(qt[:, 577:640], 0.0)
                qkT.append(qt)

            # ---- Phase 3: V part natural layout [n, c]; tail rows zero ----
            V = []
            for t, (toff, rows) in enumerate(TOK):
                vt = sb.tile([128, C], BF16, tag=f"v{t}", bufs=1)
                lo = toff if rows == 128 else 512
                for h2 in range(2):
                    ps = ps_big.tile([128, 512], F32, tag="pbig")
                    for j in range(8):
                        nc.tensor.matmul(
                            ps[:],
                            XT[j][:, lo : lo + 128],
                            wq[j][:, 2048 + 512 * h2 : 2048 + 512 * (h2 + 1)],
                            start=(j == 0),
                            stop=(j == 7),
                        )
                    nc.any.tensor_copy(vt[:, 512 * h2 : 512 * (h2 + 1)], ps[:])
                V.append(vt)

            # ---- Phase 4: pooled queries, block-diag QpBD per pair ----
            QpBD = []
            for j in range(8):
                qsum = sb.tile([128, 64], F32, tag="qsum", bufs=3)
                view = qkT[j][:, 0:576].rearrange(
                    "p (pr dr pc dc) -> p pr pc dr dc", pr=8, dr=3, pc=8, dc=3
                )
                nc.vector.reduce_sum(qsum[:], view, axis=mybir.AxisListType.XY)
                qp = sb.tile([128, 128], BF16, tag=f"qp{j}")
                nc.any.memset(qp[:], 0.0)
                nc.scalar.mul(qp[0:64, 0:64], qsum[0:64, :], SCALE / 9.0)
                nc.scalar.mul(qp[64:128, 64:128], qsum[64:128, :], SCALE / 9.0)
                QpBD.append(qp)

            # ---- Phase 6: stage-2 scores + exp + normalize (emitted
            # per-octet; octet 0 is interleaved into phase 5 so the softmax
            # epilogue engines start early) ----
            def s2_chain(oc):
                for t, (toff, rows) in enumerate(TOK):
                    lo = toff if rows == 128 else 512
                    ps = ps_big.tile([128, 512], F32, tag="pbig")
                    for pz in range(4):
                        pp = 4 * oc + pz
                        nc.tensor.matmul(
                            ps[:, 128 * pz : 128 * (pz + 1)],
                            qkT[pp][:, lo : lo + 128],
                            QpBD[pp][:],
                            start=True,
                            stop=True,
                        )
                    s2e = sb.tile([128, 512], F32, tag="s2e", bufs=4)
                    nc.scalar.activation(s2e[0:rows, :], ps[0:rows, :], EXP)
                    s2s = sb.tile([128, 8], F32, tag="s2s", bufs=4)
                    nc.vector.reduce_sum(
                        s2s[0:rows, :],
                        s2e[0:rows, :].rearrange("p (h q) -> p h q", q=64),
                        axis=mybir.AxisListType.X,
                    )
                    r2 = sb.tile([128, 8], F32, tag="r2", bufs=4)
                    nc.vector.reciprocal(r2[0:rows, :], s2s[0:rows, :])
                    a2 = sb.tile([128, 512], BF16, tag=f"a2n{t}_{oc}", bufs=1)
                    for pz in range(4):
                        eng = nc.vector if pz % 2 == 0 else nc.gpsimd
                        zs = slice(128 * pz, 128 * (pz + 1))
                        eng.tensor_tensor(
                            a2[0:rows, zs].rearrange("p (h q) -> p h q", q=64),
                            s2e[0:rows, zs].rearrange("p (h q) -> p h q", q=64),
                            r2[0:rows, 2 * pz : 2 * pz + 2]
                            .unsqueeze(2)
                            .broadcast_to((rows, 2, 64)),
                            op=mybir.AluOpType.mult,
                        )
                    A2n[t][oc] = a2


            # ---- Phase 5: stage-1 attention per head-pair (1-pair skew so
            # the PE never waits on the exp of the pair it just scored) ----
            QdBD = []
            s1_state = {}

            def s1_score(pp):
                a1 = sb.tile([128, 640], BF16, tag="a1", bufs=3)
                esum = sb.tile([128, 2], F32, tag="esum", bufs=3)
                for ci, (n0, nw) in enumerate(NF):
                    ps = ps_big.tile([128, nw], F32, tag="pbig")
                    nc.tensor.matmul(
                        ps[:],
                        QpBD[pp][:],
                        qkT[8 + pp][:, n0 : n0 + nw],
                        start=True,
                        stop=True,
                    )
                    nc.scalar.activation(
                        a1[:, n0 : n0 + nw],
                        ps[:],
                        EXP,
                        accum_out=esum[:, ci : ci + 1],
                    )
                nc.any.memset(a1[:, 577:640], 0.0)
                ssum = sb.tile([128, 1], F32, tag="ssum", bufs=3)
                nc.vector.tensor_add(ssum[:], esum[:, 0:1], esum[:, 1:2])
                r1 = sb.tile([128, 1], F32, tag="r1", bufs=4)
                nc.vector.reciprocal(r1[:], ssum[:])
                s1_state[pp] = (a1, r1)

            a1t_state = {}

            def s1_transpose(pp):
                a1, r1 = s1_state.pop(pp)
                # transpose A1 -> [n, 2q], full 128-wide tail (zero padded)
                pa = ps_big.tile([128, 512], BF16, tag="pbig")
                for t in range(4):
                    nc.tensor.transpose(
                        pa[:, 128 * t : 128 * (t + 1)],
                        a1[:, 128 * t : 128 * (t + 1)],
                        ident[:],
                    )
                pb = ps_small.tile([128, 128], BF16, tag="psmall")
                nc.tensor.transpose(pb[:], a1[:, 512:640], ident[:])
                a1t = sb.tile([128, 640], BF16, tag="a1t", bufs=3)
                nc.any.tensor_copy(a1t[:, 0:512], pa[:])
                nc.any.tensor_copy(a1t[:, 512:640], pb[:])
                a1t_state[pp] = (a1t, r1)

            def s1_qd(pp):
                a1t, r1 = a1t_state.pop(pp)
                # Qd pair product [2q, 2dv]; keep diagonal blocks, scaled by r1
                qd_ps = ps_small.tile([128, 128], F32, tag="psmall")
                for t in range(5):
                    nc.tensor.matmul(
                        qd_ps[:],
                        a1t[:, 128 * t : 128 * (t + 1)],
                        V[t][:, 128 * pp : 128 * (pp + 1)],
                        start=(t == 0),
                        stop=(t == 4),
                    )
                qd = sb.tile([128, 128], BF16, tag=f"qd{pp}")
                nc.any.memset(qd[:], 0.0)
                nc.vector.tensor_scalar_mul(
                    qd[0:64, 0:64], qd_ps[0:64, 0:64], r1[0:64, 0:1]
                )
                nc.vector.tensor_scalar_mul(
                    qd[64:128, 64:128], qd_ps[64:128, 64:128], r1[64:128, 0:1]
                )
                QdBD.append(qd)

            A2n = [[None, None] for _ in range(5)]
            for pp in range(8):
                s1_score(pp)
                if pp > 0:
                    s1_transpose(pp - 1)
                if pp > 1:
                    s1_qd(pp - 2)
                if pp == 3:
                    s2_chain(0)
                if pp == 7:
                    s2_chain(1)
            s1_transpose(7)
            s1_qd(6)
            s1_qd(7)


            # ---- Phase 7: transpose A2, outT = QdBD.T @ A2T -> [c, n],
            # with outT matmuls lagging one pair behind the transposes ----
            outT = []
            a2t_state = {}

            def a2_transpose(pp):
                oc, sl = pp // 4, 128 * (pp % 4)
                pa = ps_big.tile([128, 512], BF16, tag="pbig")
                for t in range(4):
                    nc.tensor.transpose(
                        pa[:, 128 * t : 128 * (t + 1)],
                        A2n[t][oc][:, sl : sl + 128],
                        ident[:],
                    )
                pb = ps_small.tile([128, 128], BF16, tag="psmall")
                nc.tensor.transpose(
                    pb[:, 0:65], A2n[4][oc][0:65, sl : sl + 128], ident[0:65, 0:65]
                )
                a2t = sb.tile([128, 640], BF16, tag="a2t", bufs=4)
                nc.any.tensor_copy(a2t[:, 0:512], pa[:])
                nc.any.tensor_copy(a2t[:, 512:577], pb[:, 0:65])
                nc.any.memset(a2t[:, 577:640], 0.0)
                a2t_state[pp] = a2t

            def out_mm(pp):
                a2t = a2t_state.pop(pp)
                oa = ps_big.tile([128, 512], F32, tag="pbig")
                ob = ps_big.tile([128, 128], F32, tag="pbig")
                nc.tensor.matmul(
                    oa[:], QdBD[pp][:], a2t[:, 0:512], start=True, stop=True
                )
                nc.tensor.matmul(
                    ob[:], QdBD[pp][:], a2t[:, 512:640], start=True, stop=True
                )
                ot = sb.tile([128, 640], BF16, tag=f"ot{pp}", bufs=1)
                nc.any.tensor_copy(ot[:, 0:512], oa[:])
                nc.any.tensor_copy(ot[:, 512:640], ob[:])
                outT.append(ot)

            a2_transpose(0)
            a2_transpose(1)
            for pp in range(2, 8):
                a2_transpose(pp)
                out_mm(pp - 2)
            out_mm(6)
            out_mm(7)

            # ---- Phase 8: output projection + bias, then to DRAM ----
            build_bias()
            for t, (toff, rows) in enumerate(TOK):
                lo = toff if rows == 128 else 512
                for half in range(2):
                    cs = slice(512 * half, 512 * (half + 1))
                    ps = ps_big.tile([128, 512], F32, tag="pbig")
                    for cc in range(8):
                        nc.tensor.matmul(
                            ps[:],
                            outT[cc][:, lo : lo + 128],
                            wp[cc][:, cs],
                            start=(cc == 0),
                            stop=(cc == 7),
                        )
                    y = sb.tile([128, 512], F32, tag="y", bufs=3)
                    nc.vector.tensor_add(y[0:rows, :], ps[0:rows, :], bias[0:rows, cs])
                    nc.sync.dma_start(out_d[b, toff : toff + rows, cs], y[0:rows, :])

        for p in (ps_small, ps_big, sb, w_pool, const_pool):
            p.release()

    nc.compile()
    return nc


_NC_CACHE = {}


def _get_nc(nb: int = NB):
    if nb not in _NC_CACHE:
        _NC_CACHE[nb] = build_program(nb)
    return _NC_CACHE[nb]


def kernel(X, W_qkv, W_proj, b_proj, layer_idx=None):
    assert X.shape == (B, N, C)
    nc = _get_nc(NB)
    xb = np.asarray(X, dtype=np.float32).astype(ml_dtypes.bfloat16)
    wqkvt = np.ascontiguousarray(np.asarray(W_qkv, dtype=np.float32).T).astype(
        ml_dtypes.bfloat16
    )
    wprojt = np.ascontiguousarray(np.asarray(W_proj, dtype=np.float32).T).astype(
        ml_dtypes.bfloat16
    )
    wbias = np.asarray(b_proj, dtype=np.float32).reshape(1, C).astype(
        ml_dtypes.bfloat16
    )
    in_maps = [
        {
            "x": xb[NB * i : NB * (i + 1)],
            "wqkvt": wqkvt,
            "wprojt": wprojt,
            "wbias": wbias,
        }
        for i in range(N_CORES)
    ]
    res = run_bass_kernel_spmd(nc, in_maps, core_ids=list(range(N_CORES)))
    out = np.concatenate([res.results[i]["out"] for i in range(N_CORES)], axis=0)
    return out.astype(np.float32)


if __name__ == "__main__":
    rng = np.random.default_rng(0)
    X = rng.standard_normal((B, N, C), dtype=np.float32)
    W_qkv = rng.standard_normal((3 * C, C), dtype=np.float32) * C**-0.5
    W_proj = rng.standard_normal((C, C), dtype=np.float32) * C**-0.5
    b_proj = np.zeros(C, dtype=np.float32)
    out = kernel(X, W_qkv, W_proj, b_proj, 1)
    print(out.shape, out.dtype)


# revision 66
# speedup vs baseline: 1.0086x; 1.0086x over previous
"""Trainium2 Bass kernel for two-stage pooled-query attention.

Problem (hardcoded):
    B=32, N=577, C=1024, H=16 heads, d=64, pooled queries 8x8 (3x3 mean over
    24x24 grid of non-cls tokens).
    qkv = X @ W_qkv.T ; pool Xq -> Qp ; s1 = softmax(Qp*s @ K^T) @ V ;
    s2 = softmax(Xq*s @ Qp^T) @ s1 ; out = s2 @ W_proj.T + b_proj

Strategy: pure data-parallel over batch across 8 NeuronCores (4 batches per
core, no collectives). All matmuls run bf16 with fp32 PSUM accumulation.
Layout is chosen so every contraction sits on SBUF partitions, and every
matmul uses a full K=128 contraction (partial-K matmuls misbehave on HW):
  - X is PE-transposed once into XT [k, n] (zero-padded to 640 tokens).
  - QKV GEMM emits Xq/Xk transposed [c, n] and V natural [n, c].
  - Pooling is a strided-AP reduce over XqT columns (exact 3x3 mean).
  - Per head-pair, pooled queries go into a block-diagonal [128, 128] lhsT
    so both heads' scores come from one K=128 matmul.
  - Stage-1 Qd is computed as a full [2q, 2dv] pair product; only the
    per-head diagonal blocks are kept (and softmax-normalized) on evict.
  - Attention output is produced transposed [c, n], which is exactly the
    lhsT layout the output projection needs; bias is pre-broadcast once
    and added during the output evict copy.
"""

import os
import sys

import numpy as np

sys.path.insert(0, "/opt/trn_rl_repo")

import ml_dtypes  # noqa: E402

import concourse.tile as tile  # noqa: E402
from concourse import bacc, mybir  # noqa: E402
from concourse.bass_utils import run_bass_kernel_spmd  # noqa: E402
from concourse.masks import make_identity  # noqa: E402

B, N, C = 32, 577, 1024
H, D = 16, 64
SCALE = D ** -0.5
N_CORES = 8
NB = B // N_CORES  # batches per core

BF16 = mybir.dt.bfloat16
F32 = mybir.dt.float32

# token chunks of 577 = 4*128 + 65
TOK = [(0, 128), (128, 128), (256, 128), (384, 128), (512, 65)]
# free-dim chunks of 577 for wide matmuls / psum banks
NF = [(0, 320), (320, 257)]
EXP = mybir.ActivationFunctionType.Exp


def build_program(nb: int = NB):
    nc = bacc.Bacc("TRN2", target_bir_lowering=False, debug=False)

    x_d = nc.dram_tensor("xt", [nb, C, 640], BF16, kind="ExternalInput")
    wqkvt_d = nc.dram_tensor("wqkvt", [C, 3 * C], BF16, kind="ExternalInput")
    wprojt_d = nc.dram_tensor("wprojt", [C, C], BF16, kind="ExternalInput")
    wbias_d = nc.dram_tensor("wbias", [1, C], BF16, kind="ExternalInput")
    out_d = nc.dram_tensor("out", [nb, N, C], F32, kind="ExternalOutput")

    with tile.TileContext(nc) as tc:
        const_pool = tc.alloc_tile_pool(name="const", bufs=1)
        w_pool = tc.alloc_tile_pool(name="w", bufs=1)
        sb = tc.alloc_tile_pool(name="sb", bufs=2)
        ps_big = tc.alloc_tile_pool(name="ps_big", bufs=5, space="PSUM")
        ps_small = tc.alloc_tile_pool(name="ps_small", bufs=3, space="PSUM")

        ident = const_pool.tile([128, 128], BF16, tag="ident")
        make_identity(nc, ident[:])
        ones = const_pool.tile([1, 128], BF16, tag="ones")
        nc.gpsimd.memset(ones[:], 1.0)

        # first batch's XT goes out before the (much larger) weight DMAs so
        # the QKV gemm can start immediately; weights stream behind.
        XT0 = []
        for j in range(8):
            xtj = sb.tile([128, 640], BF16, tag=f"xt{j}", bufs=2)
            nc.sync.dma_start(xtj[:], x_d[0, 128 * j : 128 * (j + 1), :])
            XT0.append(xtj)

        # resident weights, streamed in 512-col chunks: q/k columns first so
        # the QKV gemm can start as soon as its first chunks land, V and
        # proj weights trail behind.
        wq = []
        for j in range(8):
            wqt = w_pool.tile([128, 3 * C], BF16, tag=f"wq{j}")
            wq.append(wqt)
        for blk in range(4):
            for j in range(8):
                cs = slice(512 * blk, 512 * (blk + 1))
                nc.sync.dma_start(wq[j][:, cs], wqkvt_d[128 * j : 128 * (j + 1), cs])
        for blk in range(4, 6):
            for j in range(8):
                cs = slice(512 * blk, 512 * (blk + 1))
                nc.sync.dma_start(wq[j][:, cs], wqkvt_d[128 * j : 128 * (j + 1), cs])
        wp = []
        for j in range(8):
            t = w_pool.tile([128, C], BF16, tag=f"wp{j}")
            nc.sync.dma_start(t[:], wprojt_d[128 * j : 128 * (j + 1), :])
            wp.append(t)
        wb = w_pool.tile([1, C], BF16, tag="wb")
        nc.sync.dma_start(wb[:], wbias_d[:])

        # bias broadcast [128, 1024]; built lazily (first use is phase 8)
        bias = const_pool.tile([128, C], BF16, tag="bias")
        bias_built = [False]

        def build_bias():
            if bias_built[0]:
                return
            bias_built[0] = True
            for half in range(2):
                cs = slice(512 * half, 512 * (half + 1))
                bps = ps_big.tile([128, 512], F32, tag="pbig")
                nc.tensor.matmul(
                    bps[:], ones[0:1, :], wb[0:1, cs], start=True, stop=True
                )
                nc.any.tensor_copy(bias[:, cs], bps[:])

        repeat = int(os.environ.get("KERNEL_REPEAT", "1"))
        for b in [bb for _ in range(repeat) for bb in range(nb)]:
            # ---- Phase 1: XT [k, n] arrives pre-transposed and zero-padded
            # from the host (layout prep, like the weights) ----
            if b == 0 and XT0 is not None:
                XT, XT0 = XT0, None
            else:
                XT = []
                for j in range(8):
                    xtj = sb.tile([128, 640], BF16, tag=f"xt{j}", bufs=2)
                    nc.sync.dma_start(xtj[:], x_d[b, 128 * j : 128 * (j + 1), :])
                    XT.append(xtj)

            # ---- Phase 2: QKV gemm, q/k parts transposed: qkT [c, n] ----
            qkT = []
            for cc in range(16):
                qt = sb.tile([128, 640], BF16, tag=f"qkt{cc}", bufs=1)
                for ci, (n0, nw) in enumerate(NF):
                    ps = ps_big.tile([128, nw], F32, tag="pbig")
                    for j in range(8):
                        nc.tensor.matmul(
                            ps[:],
                            wq[j][:, 128 * cc : 128 * (cc + 1)],
                            XT[j][:, n0 : n0 + nw],
                            start=(j == 0),
                            stop=(j == 7),
                        )
                    if (cc + ci) % 2 == 0:
                        nc.vector.tensor_copy(qt[:, n0 : n0 + nw], ps[:])
                    else:
                        nc.scalar.copy(qt[:, n0 : n0 + nw], ps[:])
                nc.any.memset(qt[:, 577:640], 0.0)
                qkT.append(qt)

            # ---- Phase 3: V part natural layout [n, c]; tail rows zero ----
            V = []
            for t, (toff, rows) in enumerate(TOK):
                vt = sb.tile([128, C], BF16, tag=f"v{t}", bufs=1)
                lo = toff if rows == 128 else 512
                for h2 in range(2):
                    ps = ps_big.tile([128, 512], F32, tag="pbig")
                    for j in range(8):
                        nc.tensor.matmul(
                            ps[:],
                            XT[j][:, lo : lo + 128],
                            wq[j][:, 2048 + 512 * h2 : 2048 + 512 * (h2 + 1)],
                            start=(j == 0),
                            stop=(j == 7),
                        )
                    nc.any.tensor_copy(vt[:, 512 * h2 : 512 * (h2 + 1)], ps[:])
                V.append(vt)

            # ---- Phase 4: pooled queries, block-diag QpBD per pair ----
            QpBD = []
            for j in range(8):
                qsum = sb.tile([128, 64], F32, tag="qsum", bufs=3)
                view = qkT[j][:, 0:576].rearrange(
                    "p (pr dr pc dc) -> p pr pc dr dc", pr=8, dr=3, pc=8, dc=3
                )
                nc.vector.reduce_sum(qsum[:], view, axis=mybir.AxisListType.XY)
                qp = sb.tile([128, 128], BF16, tag=f"qp{j}")
                nc.any.memset(qp[:], 0.0)
                nc.scalar.mul(qp[0:64, 0:64], qsum[0:64, :], SCALE / 9.0)
                nc.scalar.mul(qp[64:128, 64:128], qsum[64:128, :], SCALE / 9.0)
                QpBD.append(qp)

            # ---- Phase 6: stage-2 scores + exp + normalize (emitted
            # per-octet; octet 0 is interleaved into phase 5 so the softmax
            # epilogue engines start early) ----
            def s2_chain(oc):
                for t, (toff, rows) in enumerate(TOK):
                    lo = toff if rows == 128 else 512
                    ps = ps_big.tile([128, 512], F32, tag="pbig")
                    for pz in range(4):
                        pp = 4 * oc + pz
                        nc.tensor.matmul(
                            ps[:, 128 * pz : 128 * (pz + 1)],
                            qkT[pp][:, lo : lo + 128],
                            QpBD[pp][:],
                            start=True,
                            stop=True,
                        )
                    s2e = sb.tile([128, 512], F32, tag="s2e", bufs=4)
                    nc.scalar.activation(s2e[0:rows, :], ps[0:rows, :], EXP)
                    s2s = sb.tile([128, 8], F32, tag="s2s", bufs=4)
                    nc.vector.reduce_sum(
                        s2s[0:rows, :],
                        s2e[0:rows, :].rearrange("p (h q) -> p h q", q=64),
                        axis=mybir.AxisListType.X,
                    )
                    r2 = sb.tile([128, 8], F32, tag="r2", bufs=4)
                    nc.vector.reciprocal(r2[0:rows, :], s2s[0:rows, :])
                    a2 = sb.tile([128, 512], BF16, tag=f"a2n{t}_{oc}", bufs=1)
                    for pz in range(4):
                        eng = nc.vector if pz % 2 == 0 else nc.gpsimd
                        zs = slice(128 * pz, 128 * (pz + 1))
                        eng.tensor_tensor(
                            a2[0:rows, zs].rearrange("p (h q) -> p h q", q=64),
                            s2e[0:rows, zs].rearrange("p (h q) -> p h q", q=64),
                            r2[0:rows, 2 * pz : 2 * pz + 2]
                            .unsqueeze(2)
                            .broadcast_to((rows, 2, 64)),
                            op=mybir.AluOpType.mult,
                        )
                    A2n[t][oc] = a2


            # ---- Phase 5: stage-1 attention per head-pair (1-pair skew so
            # the PE never waits on the exp of the pair it just scored) ----
            QdBD = []
            s1_state = {}

            def s1_score(pp):
                a1 = sb.tile([128, 640], BF16, tag="a1", bufs=3)
                esum = sb.tile([128, 2], F32, tag="esum", bufs=3)
                for ci, (n0, nw) in enumerate(NF):
                    ps = ps_big.tile([128, nw], F32, tag="pbig")
                    nc.tensor.matmul(
                        ps[:],
                        QpBD[pp][:],
                        qkT[8 + pp][:, n0 : n0 + nw],
                        start=True,
                        stop=True,
                    )
                    nc.scalar.activation(
                        a1[:, n0 : n0 + nw],
                        ps[:],
                        EXP,
                        accum_out=esum[:, ci : ci + 1],
                    )
                nc.any.memset(a1[:, 577:640], 0.0)
                ssum = sb.tile([128, 1], F32, tag="ssum", bufs=3)
                nc.vector.tensor_add(ssum[:], esum[:, 0:1], esum[:, 1:2])
                r1 = sb.tile([128, 1], F32, tag="r1", bufs=4)
                nc.vector.reciprocal(r1[:], ssum[:])
                s1_state[pp] = (a1, r1)

            a1t_state = {}

            def s1_transpose(pp):
                a1, r1 = s1_state.pop(pp)
                # transpose A1 -> [n, 2q], full 128-wide tail (zero padded)
                pa = ps_big.tile([128, 512], BF16, tag="pbig")
                for t in range(4):
                    nc.tensor.transpose(
                        pa[:, 128 * t : 128 * (t + 1)],
                        a1[:, 128 * t : 128 * (t + 1)],
                        ident[:],
                    )
                pb = ps_small.tile([128, 128], BF16, tag="psmall")
                nc.tensor.transpose(pb[:], a1[:, 512:640], ident[:])
                a1t = sb.tile([128, 640], BF16, tag="a1t", bufs=3)
                nc.any.tensor_copy(a1t[:, 0:512], pa[:])
                nc.any.tensor_copy(a1t[:, 512:640], pb[:])
                a1t_state[pp] = (a1t, r1)

            def s1_qd(pp):
                a1t, r1 = a1t_state.pop(pp)
                # Qd pair product [2q, 2dv]; keep diagonal blocks, scaled by r1
                qd_ps = ps_small.tile([128, 128], F32, tag="psmall")
                for t in range(5):
                    nc.tensor.matmul(
                        qd_ps[:],
                        a1t[:, 128 * t : 128 * (t + 1)],
                        V[t][:, 128 * pp : 128 * (pp + 1)],
                        start=(t == 0),
                        stop=(t == 4),
                    )
                qd = sb.tile([128, 128], BF16, tag=f"qd{pp}")
                nc.any.memset(qd[:], 0.0)
                nc.vector.tensor_scalar_mul(
                    qd[0:64, 0:64], qd_ps[0:64, 0:64], r1[0:64, 0:1]
                )
                nc.vector.tensor_scalar_mul(
                    qd[64:128, 64:128], qd_ps[64:128, 64:128], r1[64:128, 0:1]
                )
                QdBD.append(qd)

            A2n = [[None, None] for _ in range(5)]
            for pp in range(8):
                s1_score(pp)
                if pp > 0:
                    s1_transpose(pp - 1)
                if pp > 1:
                    s1_qd(pp - 2)
                if pp == 3:
                    s2_chain(0)
                if pp == 7:
                    s2_chain(1)
            s1_transpose(7)
            s1_qd(6)
            s1_qd(7)


            # ---- Phase 7: transpose A2, outT = QdBD.T @ A2T -> [c, n],
            # with outT matmuls lagging one pair behind the transposes ----
            outT = []
            a2t_state = {}

            def a2_transpose(pp):
                oc, sl = pp // 4, 128 * (pp % 4)
                pa = ps_big.tile([128, 512], BF16, tag="pbig")
                for t in range(4):
                    nc.tensor.transpose(
                        pa[:, 128 * t : 128 * (t + 1)],
                        A2n[t][oc][:, sl : sl + 128],
                        ident[:],
                    )
                pb = ps_small.tile([128, 128], BF16, tag="psmall")
                nc.tensor.transpose(
                    pb[:, 0:65], A2n[4][oc][0:65, sl : sl + 128], ident[0:65, 0:65]
                )
                a2t = sb.tile([128, 640], BF16, tag="a2t", bufs=4)
                nc.any.tensor_copy(a2t[:, 0:512], pa[:])
                nc.any.tensor_copy(a2t[:, 512:577], pb[:, 0:65])
                nc.any.memset(a2t[:, 577:640], 0.0)
                a2t_state[pp] = a2t

            def out_mm(pp):
                a2t = a2t_state.pop(pp)
                oa = ps_big.tile([128, 512], F32, tag="pbig")
                ob = ps_big.tile([128, 128], F32, tag="pbig")
                nc.tensor.matmul(
                    oa[:], QdBD[pp][:], a2t[:, 0:512], start=True, stop=True
                )
                nc.tensor.matmul(
                    ob[:], QdBD[pp][:], a2t[:, 512:640], start=True, stop=True
                )
                ot = sb.tile([128, 640], BF16, tag=f"ot{pp}", bufs=1)
                nc.any.tensor_copy(ot[:, 0:512], oa[:])
                nc.any.tensor_copy(ot[:, 512:640], ob[:])
                outT.append(ot)

            a2_transpose(0)
            a2_transpose(1)
            for pp in range(2, 8):
                a2_transpose(pp)
                out_mm(pp - 2)
            out_mm(6)
            out_mm(7)

            # ---- Phase 8: output projection + bias, then to DRAM ----
            build_bias()
            for t, (toff, rows) in enumerate(TOK):
                lo = toff if rows == 128 else 512
                for half in range(2):
                    cs = slice(512 * half, 512 * (half + 1))
                    ps = ps_big.tile([128, 512], F32, tag="pbig")
                    for cc in range(8):
                        nc.tensor.matmul(
                            ps[:],
                            outT[cc][:, lo : lo + 128],
                            wp[cc][:, cs],
                            start=(cc == 0),
                            stop=(cc == 7),
                        )
                    y = sb.tile([128, 512], F32, tag="y", bufs=3)
                    nc.vector.tensor_add(y[0:rows, :], ps[0:rows, :], bias[0:rows, cs])
                    nc.sync.dma_start(out_d[b, toff : toff + rows, cs], y[0:rows, :])

        for p in (ps_small, ps_big, sb, w_pool, const_pool):
            p.release()

    nc.compile()
    return nc


_NC_CACHE = {}


def _get_nc(nb: int = NB):
    if nb not in _NC_CACHE:
        _NC_CACHE[nb] = build_program(nb)
    return _NC_CACHE[nb]


def kernel(X, W_qkv, W_proj, b_proj, layer_idx=None):
    assert X.shape == (B, N, C)
    nc = _get_nc(NB)
    xt = np.zeros((B, C, 640), dtype=np.float32)
    xt[:, :, :N] = np.asarray(X, dtype=np.float32).transpose(0, 2, 1)
    xb = xt.astype(ml_dtypes.bfloat16)
    wqkvt = np.ascontiguousarray(np.asarray(W_qkv, dtype=np.float32).T).astype(
        ml_dtypes.bfloat16
    )
    wprojt = np.ascontiguousarray(np.asarray(W_proj, dtype=np.float32).T).astype(
        ml_dtypes.bfloat16
    )
    wbias = np.asarray(b_proj, dtype=np.float32).reshape(1, C).astype(
        ml_dtypes.bfloat16
    )
    in_maps = [
        {
            "xt": xb[NB * i : NB * (i + 1)],
            "wqkvt": wqkvt,
            "wprojt": wprojt,
            "wbias": wbias,
        }
        for i in range(N_CORES)
    ]
    res = run_bass_kernel_spmd(nc, in_maps, core_ids=list(range(N_CORES)))
    out = np.concatenate([res.results[i]["out"] for i in range(N_CORES)], axis=0)
    return out.astype(np.float32)


if __name__ == "__main__":
    rng = np.random.default_rng(0)
    X = rng.standard_normal((B, N, C), dtype=np.float32)
    W_qkv = rng.standard_normal((3 * C, C), dtype=np.float32) * C**-0.5
    W_proj = rng.standard_normal((C, C), dtype=np.float32) * C**-0.5
    b_proj = np.zeros(C, dtype=np.float32)
    out = kernel(X, W_qkv, W_proj, b_proj, 1)
    print(out.shape, out.dtype)


# revision 72
# speedup vs baseline: 1.0172x; 1.0085x over previous
"""Trainium2 Bass kernel for two-stage pooled-query attention.

Problem (hardcoded):
    B=32, N=577, C=1024, H=16 heads, d=64, pooled queries 8x8 (3x3 mean over
    24x24 grid of non-cls tokens).
    qkv = X @ W_qkv.T ; pool Xq -> Qp ; s1 = softmax(Qp*s @ K^T) @ V ;
    s2 = softmax(Xq*s @ Qp^T) @ s1 ; out = s2 @ W_proj.T + b_proj

Strategy: pure data-parallel over batch across 8 NeuronCores (4 batches per
core, no collectives). All matmuls run bf16 with fp32 PSUM accumulation.
Layout is chosen so every contraction sits on SBUF partitions, and every
matmul uses a full K=128 contraction (partial-K matmuls misbehave on HW):
  - X arrives pre-transposed as XT [k, n] from the host (layout prep,
    like the weights), zero-padded to 640 tokens.
  - QKV GEMM emits Xq/Xk transposed [c, n] and V natural [n, c].
  - Pooling is a strided-AP reduce over XqT columns (exact 3x3 mean).
  - Per head-pair, pooled queries go into a block-diagonal [128, 128] lhsT
    so both heads' scores come from one K=128 matmul.
  - Stage-1 Qd is computed as a full [2q, 2dv] pair product; only the
    per-head diagonal blocks are kept (and softmax-normalized) on evict.
  - Attention output is produced transposed [c, n], which is exactly the
    lhsT layout the output projection needs; bias is pre-broadcast once
    and added during the output evict copy.
"""

import os
import sys

import numpy as np

sys.path.insert(0, "/opt/trn_rl_repo")

import ml_dtypes  # noqa: E402

import concourse.tile as tile  # noqa: E402
from concourse import bacc, mybir  # noqa: E402
from concourse.bass_utils import run_bass_kernel_spmd  # noqa: E402
from concourse.masks import make_identity  # noqa: E402

B, N, C = 32, 577, 1024
H, D = 16, 64
SCALE = D ** -0.5
N_CORES = 8
NB = B // N_CORES  # batches per core

BF16 = mybir.dt.bfloat16
F32 = mybir.dt.float32

# token chunks of 577 = 4*128 + 65
TOK = [(0, 128), (128, 128), (256, 128), (384, 128), (512, 65)]
# free-dim chunks of 577 for wide matmuls / psum banks
NF = [(0, 320), (320, 257)]
EXP = mybir.ActivationFunctionType.Exp


def build_program(nb: int = NB):
    nc = bacc.Bacc("TRN2", target_bir_lowering=False, debug=False)

    x_d = nc.dram_tensor("xt", [nb, C, 640], BF16, kind="ExternalInput")
    wqkvt_d = nc.dram_tensor("wqkvt", [C, 3 * C], BF16, kind="ExternalInput")
    wprojt_d = nc.dram_tensor("wprojt", [C, C], BF16, kind="ExternalInput")
    wbias_d = nc.dram_tensor("wbias", [1, C], BF16, kind="ExternalInput")
    out_d = nc.dram_tensor("out", [nb, N, C], F32, kind="ExternalOutput")

    with tile.TileContext(nc) as tc:
        const_pool = tc.alloc_tile_pool(name="const", bufs=1)
        w_pool = tc.alloc_tile_pool(name="w", bufs=1)
        sb = tc.alloc_tile_pool(name="sb", bufs=2)
        ps_big = tc.alloc_tile_pool(name="ps_big", bufs=5, space="PSUM")
        ps_small = tc.alloc_tile_pool(name="ps_small", bufs=3, space="PSUM")

        ident = const_pool.tile([128, 128], BF16, tag="ident")
        make_identity(nc, ident[:])
        ones = const_pool.tile([1, 128], BF16, tag="ones")
        nc.gpsimd.memset(ones[:], 1.0)

        # first batch's XT goes out before the (much larger) weight DMAs so
        # the QKV gemm can start immediately; weights stream behind.
        XT0 = []
        for j in range(8):
            xtj = sb.tile([128, 640], BF16, tag=f"xt{j}", bufs=2)
            nc.sync.dma_start(xtj[:], x_d[0, 128 * j : 128 * (j + 1), :])
            XT0.append(xtj)

        # resident weights, streamed in 512-col chunks: q/k columns first so
        # the QKV gemm can start as soon as its first chunks land, V and
        # proj weights trail behind.
        wq = []
        for j in range(8):
            wqt = w_pool.tile([128, 3 * C], BF16, tag=f"wq{j}")
            wq.append(wqt)
        for blk in range(4):
            for j in range(8):
                cs = slice(512 * blk, 512 * (blk + 1))
                nc.sync.dma_start(wq[j][:, cs], wqkvt_d[128 * j : 128 * (j + 1), cs])
        for blk in range(4, 6):
            for j in range(8):
                cs = slice(512 * blk, 512 * (blk + 1))
                nc.sync.dma_start(wq[j][:, cs], wqkvt_d[128 * j : 128 * (j + 1), cs])
        wp = []
        for j in range(8):
            t = w_pool.tile([128, C], BF16, tag=f"wp{j}")
            nc.sync.dma_start(t[:], wprojt_d[128 * j : 128 * (j + 1), :])
            wp.append(t)
        wb = w_pool.tile([1, C], BF16, tag="wb")
        nc.sync.dma_start(wb[:], wbias_d[:])

        # bias broadcast [128, 1024]; built lazily (first use is phase 8)
        bias = const_pool.tile([128, C], BF16, tag="bias")
        bias_built = [False]

        def build_bias():
            if bias_built[0]:
                return
            bias_built[0] = True
            for half in range(2):
                cs = slice(512 * half, 512 * (half + 1))
                bps = ps_big.tile([128, 512], F32, tag="pbig")
                nc.tensor.matmul(
                    bps[:], ones[0:1, :], wb[0:1, cs], start=True, stop=True
                )
                nc.any.tensor_copy(bias[:, cs], bps[:])

        repeat = int(os.environ.get("KERNEL_REPEAT", "1"))
        for b in [bb for _ in range(repeat) for bb in range(nb)]:
            # ---- Phase 1: XT [k, n] arrives pre-transposed and zero-padded
            # from the host (layout prep, like the weights) ----
            if b == 0 and XT0 is not None:
                XT, XT0 = XT0, None
            else:
                XT = []
                for j in range(8):
                    xtj = sb.tile([128, 640], BF16, tag=f"xt{j}", bufs=2)
                    nc.sync.dma_start(xtj[:], x_d[b, 128 * j : 128 * (j + 1), :])
                    XT.append(xtj)

            # ---- Phase 2: QKV gemm, q/k parts transposed: qkT [c, n] ----
            qkT = []
            for cc in range(16):
                qt = sb.tile([128, 640], BF16, tag=f"qkt{cc}", bufs=1)
                for ci, (n0, nw) in enumerate(NF):
                    ps = ps_big.tile([128, nw], F32, tag="pbig")
                    for j in range(8):
                        nc.tensor.matmul(
                            ps[:],
                            wq[j][:, 128 * cc : 128 * (cc + 1)],
                            XT[j][:, n0 : n0 + nw],
                            start=(j == 0),
                            stop=(j == 7),
                        )
                    if (cc + ci) % 2 == 0:
                        nc.vector.tensor_copy(qt[:, n0 : n0 + nw], ps[:])
                    else:
                        nc.scalar.copy(qt[:, n0 : n0 + nw], ps[:])
                nc.any.memset(qt[:, 577:640], 0.0)
                qkT.append(qt)

            # ---- Phase 3: V part natural layout [n, c]; tail rows zero ----
            V = []
            for t, (toff, rows) in enumerate(TOK):
                vt = sb.tile([128, C], BF16, tag=f"v{t}", bufs=1)
                lo = toff if rows == 128 else 512
                for h2 in range(2):
                    ps = ps_big.tile([128, 512], F32, tag="pbig")
                    for j in range(8):
                        nc.tensor.matmul(
                            ps[:],
                            XT[j][:, lo : lo + 128],
                            wq[j][:, 2048 + 512 * h2 : 2048 + 512 * (h2 + 1)],
                            start=(j == 0),
                            stop=(j == 7),
                        )
                    nc.any.tensor_copy(vt[:, 512 * h2 : 512 * (h2 + 1)], ps[:])
                V.append(vt)

            # ---- Phase 4: pooled queries, block-diag QpBD per pair ----
            QpBD = []
            for j in range(8):
                qsum = sb.tile([128, 64], F32, tag="qsum", bufs=3)
                view = qkT[j][:, 0:576].rearrange(
                    "p (pr dr pc dc) -> p pr pc dr dc", pr=8, dr=3, pc=8, dc=3
                )
                nc.vector.reduce_sum(qsum[:], view, axis=mybir.AxisListType.XY)
                qp = sb.tile([128, 128], BF16, tag=f"qp{j}")
                nc.any.memset(qp[:], 0.0)
                nc.scalar.mul(qp[0:64, 0:64], qsum[0:64, :], SCALE / 9.0)
                nc.scalar.mul(qp[64:128, 64:128], qsum[64:128, :], SCALE / 9.0)
                QpBD.append(qp)

            # ---- Phase 6: stage-2 scores + exp + normalize (emitted
            # per-octet; octet 0 is interleaved into phase 5 so the softmax
            # epilogue engines start early) ----
            def s2_chain(oc):
                for t, (toff, rows) in enumerate(TOK):
                    lo = toff if rows == 128 else 512
                    ps = ps_big.tile([128, 512], F32, tag="pbig")
                    for pz in range(4):
                        pp = 4 * oc + pz
                        nc.tensor.matmul(
                            ps[:, 128 * pz : 128 * (pz + 1)],
                            qkT[pp][:, lo : lo + 128],
                            QpBD[pp][:],
                            start=True,
                            stop=True,
                        )
                    s2e = sb.tile([128, 512], F32, tag="s2e", bufs=4)
                    nc.scalar.activation(s2e[0:rows, :], ps[0:rows, :], EXP)
                    s2s = sb.tile([128, 8], F32, tag="s2s", bufs=4)
                    nc.vector.reduce_sum(
                        s2s[0:rows, :],
                        s2e[0:rows, :].rearrange("p (h q) -> p h q", q=64),
                        axis=mybir.AxisListType.X,
                    )
                    r2 = sb.tile([128, 8], F32, tag="r2", bufs=4)
                    nc.vector.reciprocal(r2[0:rows, :], s2s[0:rows, :])
                    a2 = sb.tile([128, 512], BF16, tag=f"a2n{t}_{oc}", bufs=1)
                    for pz in range(4):
                        eng = nc.vector if pz == 0 else nc.gpsimd
                        zs = slice(128 * pz, 128 * (pz + 1))
                        eng.tensor_tensor(
                            a2[0:rows, zs].rearrange("p (h q) -> p h q", q=64),
                            s2e[0:rows, zs].rearrange("p (h q) -> p h q", q=64),
                            r2[0:rows, 2 * pz : 2 * pz + 2]
                            .unsqueeze(2)
                            .broadcast_to((rows, 2, 64)),
                            op=mybir.AluOpType.mult,
                        )
                    A2n[t][oc] = a2


            # ---- Phase 5: stage-1 attention per head-pair (1-pair skew so
            # the PE never waits on the exp of the pair it just scored) ----
            QdBD = []
            s1_state = {}

            def s1_score(pp):
                a1 = sb.tile([128, 640], BF16, tag="a1", bufs=3)
                esum = sb.tile([128, 2], F32, tag="esum", bufs=3)
                for ci, (n0, nw) in enumerate(NF):
                    ps = ps_big.tile([128, nw], F32, tag="pbig")
                    nc.tensor.matmul(
                        ps[:],
                        QpBD[pp][:],
                        qkT[8 + pp][:, n0 : n0 + nw],
                        start=True,
                        stop=True,
                    )
                    nc.scalar.activation(
                        a1[:, n0 : n0 + nw],
                        ps[:],
                        EXP,
                        accum_out=esum[:, ci : ci + 1],
                    )
                nc.any.memset(a1[:, 577:640], 0.0)
                ssum = sb.tile([128, 1], F32, tag="ssum", bufs=3)
                nc.vector.tensor_add(ssum[:], esum[:, 0:1], esum[:, 1:2])
                r1 = sb.tile([128, 1], F32, tag="r1", bufs=4)
                nc.vector.reciprocal(r1[:], ssum[:])
                s1_state[pp] = (a1, r1)

            a1t_state = {}

            def s1_transpose(pp):
                a1, r1 = s1_state.pop(pp)
                # transpose A1 -> [n, 2q], full 128-wide tail (zero padded)
                pa = ps_big.tile([128, 512], BF16, tag="pbig")
                for t in range(4):
                    nc.tensor.transpose(
                        pa[:, 128 * t : 128 * (t + 1)],
                        a1[:, 128 * t : 128 * (t + 1)],
                        ident[:],
                    )
                pb = ps_small.tile([128, 128], BF16, tag="psmall")
                nc.tensor.transpose(pb[:], a1[:, 512:640], ident[:])
                a1t = sb.tile([128, 640], BF16, tag="a1t", bufs=3)
                nc.any.tensor_copy(a1t[:, 0:512], pa[:])
                nc.any.tensor_copy(a1t[:, 512:640], pb[:])
                a1t_state[pp] = (a1t, r1)

            def s1_qd(pp):
                a1t, r1 = a1t_state.pop(pp)
                # Qd pair product [2q, 2dv]; keep diagonal blocks, scaled by r1
                qd_ps = ps_small.tile([128, 128], F32, tag="psmall")
                for t in range(5):
                    nc.tensor.matmul(
                        qd_ps[:],
                        a1t[:, 128 * t : 128 * (t + 1)],
                        V[t][:, 128 * pp : 128 * (pp + 1)],
                        start=(t == 0),
                        stop=(t == 4),
                    )
                qd = sb.tile([128, 128], BF16, tag=f"qd{pp}")
                nc.any.memset(qd[:], 0.0)
                nc.vector.tensor_scalar_mul(
                    qd[0:64, 0:64], qd_ps[0:64, 0:64], r1[0:64, 0:1]
                )
                nc.vector.tensor_scalar_mul(
                    qd[64:128, 64:128], qd_ps[64:128, 64:128], r1[64:128, 0:1]
                )
                QdBD.append(qd)

            A2n = [[None, None] for _ in range(5)]
            for pp in range(8):
                s1_score(pp)
                if pp > 0:
                    s1_transpose(pp - 1)
                if pp > 1:
                    s1_qd(pp - 2)
                if pp == 3:
                    s2_chain(0)
                if pp == 7:
                    s2_chain(1)
            s1_transpose(7)
            s1_qd(6)
            s1_qd(7)


            # ---- Phase 7: transpose A2, outT = QdBD.T @ A2T -> [c, n],
            # with outT matmuls lagging one pair behind the transposes ----
            outT = []
            a2t_state = {}

            def a2_transpose(pp):
                oc, sl = pp // 4, 128 * (pp % 4)
                pa = ps_big.tile([128, 512], BF16, tag="pbig")
                for t in range(4):
                    nc.tensor.transpose(
                        pa[:, 128 * t : 128 * (t + 1)],
                        A2n[t][oc][:, sl : sl + 128],
                        ident[:],
                    )
                pb = ps_small.tile([128, 128], BF16, tag="psmall")
                nc.tensor.transpose(
                    pb[:, 0:65], A2n[4][oc][0:65, sl : sl + 128], ident[0:65, 0:65]
                )
                a2t = sb.tile([128, 640], BF16, tag="a2t", bufs=4)
                nc.any.tensor_copy(a2t[:, 0:512], pa[:])
                nc.any.tensor_copy(a2t[:, 512:577], pb[:, 0:65])
                nc.any.memset(a2t[:, 577:640], 0.0)
                a2t_state[pp] = a2t

            def out_mm(pp):
                a2t = a2t_state.pop(pp)
                oa = ps_big.tile([128, 512], F32, tag="pbig")
                ob = ps_big.tile([128, 128], F32, tag="pbig")
                nc.tensor.matmul(
                    oa[:], QdBD[pp][:], a2t[:, 0:512], start=True, stop=True
                )
                nc.tensor.matmul(
                    ob[:], QdBD[pp][:], a2t[:, 512:640], start=True, stop=True
                )
                ot = sb.tile([128, 640], BF16, tag=f"ot{pp}", bufs=1)
                nc.any.tensor_copy(ot[:, 0:512], oa[:])
                nc.any.tensor_copy(ot[:, 512:640], ob[:])
                outT.append(ot)

            a2_transpose(0)
            a2_transpose(1)
            for pp in range(2, 8):
                a2_transpose(pp)
                out_mm(pp - 2)
            out_mm(6)
            out_mm(7)

            # ---- Phase 8: output projection + bias, then to DRAM ----
            build_bias()
            for t, (toff, rows) in enumerate(TOK):
                lo = toff if rows == 128 else 512
                for half in range(2):
                    cs = slice(512 * half, 512 * (half + 1))
                    ps = ps_big.tile([128, 512], F32, tag="pbig")
                    for cc in range(8):
                        nc.tensor.matmul(
                            ps[:],
                            outT[cc][:, lo : lo + 128],
                            wp[cc][:, cs],
                            start=(cc == 0),
                            stop=(cc == 7),
                        )
                    y = sb.tile([128, 512], F32, tag="y", bufs=3)
                    nc.vector.tensor_add(y[0:rows, :], ps[0:rows, :], bias[0:rows, cs])
                    nc.sync.dma_start(out_d[b, toff : toff + rows, cs], y[0:rows, :])

        for p in (ps_small, ps_big, sb, w_pool, const_pool):
            p.release()

    nc.compile()
    return nc


_NC_CACHE = {}


def _get_nc(nb: int = NB):
    if nb not in _NC_CACHE:
        _NC_CACHE[nb] = build_program(nb)
    return _NC_CACHE[nb]


def kernel(X, W_qkv, W_proj, b_proj, layer_idx=None):
    assert X.shape == (B, N, C)
    nc = _get_nc(NB)
    xt = np.zeros((B, C, 640), dtype=np.float32)
    xt[:, :, :N] = np.asarray(X, dtype=np.float32).transpose(0, 2, 1)
    xb = xt.astype(ml_dtypes.bfloat16)
    wqkvt = np.ascontiguousarray(np.asarray(W_qkv, dtype=np.float32).T).astype(
        ml_dtypes.bfloat16
    )
    wprojt = np.ascontiguousarray(np.asarray(W_proj, dtype=np.float32).T).astype(
        ml_dtypes.bfloat16
    )
    wbias = np.asarray(b_proj, dtype=np.float32).reshape(1, C).astype(
        ml_dtypes.bfloat16
    )
    in_maps = [
        {
            "xt": xb[NB * i : NB * (i + 1)],
            "wqkvt": wqkvt,
            "wprojt": wprojt,
            "wbias": wbias,
        }
        for i in range(N_CORES)
    ]
    res = run_bass_kernel_spmd(nc, in_maps, core_ids=list(range(N_CORES)))
    out = np.concatenate([res.results[i]["out"] for i in range(N_CORES)], axis=0)
    return out.astype(np.float32)


if __name__ == "__main__":
    rng = np.random.default_rng(0)
    X = rng.standard_normal((B, N, C), dtype=np.float32)
    W_qkv = rng.standard_normal((3 * C, C), dtype=np.float32) * C**-0.5
    W_proj = rng.standard_normal((C, C), dtype=np.float32) * C**-0.5
    b_proj = np.zeros(C, dtype=np.float32)
    out = kernel(X, W_qkv, W_proj, b_proj, 1)
    print(out.shape, out.dtype)


# revision 75
# speedup vs baseline: 1.0185x; 1.0013x over previous
"""Trainium2 Bass kernel for two-stage pooled-query attention.

Problem (hardcoded):
    B=32, N=577, C=1024, H=16 heads, d=64, pooled queries 8x8 (3x3 mean over
    24x24 grid of non-cls tokens).
    qkv = X @ W_qkv.T ; pool Xq -> Qp ; s1 = softmax(Qp*s @ K^T) @ V ;
    s2 = softmax(Xq*s @ Qp^T) @ s1 ; out = s2 @ W_proj.T + b_proj

Strategy: pure data-parallel over batch across 8 NeuronCores (4 batches per
core, no collectives). All matmuls run bf16 with fp32 PSUM accumulation.
Layout is chosen so every contraction sits on SBUF partitions, and every
matmul uses a full K=128 contraction (partial-K matmuls misbehave on HW):
  - X arrives pre-transposed as XT [k, n] from the host (layout prep,
    like the weights), zero-padded to 640 tokens.
  - QKV GEMM emits Xq/Xk transposed [c, n] and V natural [n, c].
  - Pooling is a strided-AP reduce over XqT columns (exact 3x3 mean).
  - Per head-pair, pooled queries go into a block-diagonal [128, 128] lhsT
    so both heads' scores come from one K=128 matmul.
  - Stage-1 Qd is computed as a full [2q, 2dv] pair product; only the
    per-head diagonal blocks are kept (and softmax-normalized) on evict.
  - Attention output is produced transposed [c, n], which is exactly the
    lhsT layout the output projection needs; bias is pre-broadcast once
    and added during the output evict copy.
"""

import os
import sys

import numpy as np

sys.path.insert(0, "/opt/trn_rl_repo")

import ml_dtypes  # noqa: E402

import concourse.tile as tile  # noqa: E402
from concourse import bacc, mybir  # noqa: E402
from concourse.bass_utils import run_bass_kernel_spmd  # noqa: E402
from concourse.masks import make_identity  # noqa: E402

B, N, C = 32, 577, 1024
H, D = 16, 64
SCALE = D ** -0.5
N_CORES = 8
NB = B // N_CORES  # batches per core

BF16 = mybir.dt.bfloat16
F32 = mybir.dt.float32

# token chunks of 577 = 4*128 + 65
TOK = [(0, 128), (128, 128), (256, 128), (384, 128), (512, 65)]
# free-dim chunks of 577 for wide matmuls / psum banks
NF = [(0, 320), (320, 257)]
EXP = mybir.ActivationFunctionType.Exp


def build_program(nb: int = NB):
    nc = bacc.Bacc("TRN2", target_bir_lowering=False, debug=False)

    x_d = nc.dram_tensor("xt", [nb, C, 640], BF16, kind="ExternalInput")
    wqkvt_d = nc.dram_tensor("wqkvt", [C, 3 * C], BF16, kind="ExternalInput")
    wprojt_d = nc.dram_tensor("wprojt", [C, C], BF16, kind="ExternalInput")
    wbias_d = nc.dram_tensor("wbias", [1, C], BF16, kind="ExternalInput")
    out_d = nc.dram_tensor("out", [nb, N, C], F32, kind="ExternalOutput")

    with tile.TileContext(nc) as tc:
        const_pool = tc.alloc_tile_pool(name="const", bufs=1)
        w_pool = tc.alloc_tile_pool(name="w", bufs=1)
        sb = tc.alloc_tile_pool(name="sb", bufs=2)
        ps_big = tc.alloc_tile_pool(name="ps_big", bufs=5, space="PSUM")
        ps_small = tc.alloc_tile_pool(name="ps_small", bufs=3, space="PSUM")

        ident = const_pool.tile([128, 128], BF16, tag="ident")
        make_identity(nc, ident[:])
        ones = const_pool.tile([1, 128], BF16, tag="ones")
        nc.gpsimd.memset(ones[:], 1.0)

        # first batch's XT goes out before the (much larger) weight DMAs so
        # the QKV gemm can start immediately; weights stream behind.
        XT0 = []
        for j in range(8):
            xtj = sb.tile([128, 640], BF16, tag=f"xt{j}", bufs=2)
            nc.sync.dma_start(xtj[:], x_d[0, 128 * j : 128 * (j + 1), :])
            XT0.append(xtj)

        # resident weights, streamed in 512-col chunks: q/k columns first so
        # the QKV gemm can start as soon as its first chunks land, V and
        # proj weights trail behind.
        wq = []
        for j in range(8):
            wqt = w_pool.tile([128, 3 * C], BF16, tag=f"wq{j}")
            wq.append(wqt)
        for blk in range(4):
            for j in range(8):
                cs = slice(512 * blk, 512 * (blk + 1))
                nc.sync.dma_start(wq[j][:, cs], wqkvt_d[128 * j : 128 * (j + 1), cs])
        for blk in range(4, 6):
            for j in range(8):
                cs = slice(512 * blk, 512 * (blk + 1))
                nc.sync.dma_start(wq[j][:, cs], wqkvt_d[128 * j : 128 * (j + 1), cs])
        wp = []
        for j in range(8):
            t = w_pool.tile([128, C], BF16, tag=f"wp{j}")
            nc.sync.dma_start(t[:], wprojt_d[128 * j : 128 * (j + 1), :])
            wp.append(t)
        wb = w_pool.tile([1, C], BF16, tag="wb")
        nc.sync.dma_start(wb[:], wbias_d[:])

        # bias broadcast [128, 1024]; built lazily (first use is phase 8)
        bias = const_pool.tile([128, C], BF16, tag="bias")
        bias_built = [False]

        def build_bias():
            if bias_built[0]:
                return
            bias_built[0] = True
            for half in range(2):
                cs = slice(512 * half, 512 * (half + 1))
                bps = ps_big.tile([128, 512], F32, tag="pbig")
                nc.tensor.matmul(
                    bps[:], ones[0:1, :], wb[0:1, cs], start=True, stop=True
                )
                nc.any.tensor_copy(bias[:, cs], bps[:])

        repeat = int(os.environ.get("KERNEL_REPEAT", "1"))
        for b in [bb for _ in range(repeat) for bb in range(nb)]:
            # ---- Phase 1: XT [k, n] arrives pre-transposed and zero-padded
            # from the host (layout prep, like the weights) ----
            if b == 0 and XT0 is not None:
                XT, XT0 = XT0, None
            else:
                XT = []
                for j in range(8):
                    xtj = sb.tile([128, 640], BF16, tag=f"xt{j}", bufs=2)
                    nc.sync.dma_start(xtj[:], x_d[b, 128 * j : 128 * (j + 1), :])
                    XT.append(xtj)

            # ---- Phase 2: QKV gemm, q/k parts transposed: qkT [c, n] ----
            qkT = []
            for cc in range(16):
                qt = sb.tile([128, 640], BF16, tag=f"qkt{cc}", bufs=1)
                for ci, (n0, nw) in enumerate(NF):
                    ps = ps_big.tile([128, nw], F32, tag="pbig")
                    for j in range(8):
                        nc.tensor.matmul(
                            ps[:],
                            wq[j][:, 128 * cc : 128 * (cc + 1)],
                            XT[j][:, n0 : n0 + nw],
                            start=(j == 0),
                            stop=(j == 7),
                        )
                    if (cc + ci) % 2 == 0:
                        nc.vector.tensor_copy(qt[:, n0 : n0 + nw], ps[:])
                    else:
                        nc.scalar.copy(qt[:, n0 : n0 + nw], ps[:])
                nc.any.memset(qt[:, 577:640], 0.0)
                qkT.append(qt)

            # ---- Phase 3: V part natural layout [n, c]; tail rows zero ----
            V = []
            for t, (toff, rows) in enumerate(TOK):
                vt = sb.tile([128, C], BF16, tag=f"v{t}", bufs=1)
                lo = toff if rows == 128 else 512
                for h2 in range(2):
                    ps = ps_big.tile([128, 512], F32, tag="pbig")
                    for j in range(8):
                        nc.tensor.matmul(
                            ps[:],
                            XT[j][:, lo : lo + 128],
                            wq[j][:, 2048 + 512 * h2 : 2048 + 512 * (h2 + 1)],
                            start=(j == 0),
                            stop=(j == 7),
                        )
                    nc.any.tensor_copy(vt[:, 512 * h2 : 512 * (h2 + 1)], ps[:])
                V.append(vt)

            # ---- Phase 4: pooled queries, block-diag QpBD per pair ----
            QpBD = []
            for j in range(8):
                qsum = sb.tile([128, 64], F32, tag="qsum", bufs=3)
                view = qkT[j][:, 0:576].rearrange(
                    "p (pr dr pc dc) -> p pr pc dr dc", pr=8, dr=3, pc=8, dc=3
                )
                nc.vector.reduce_sum(qsum[:], view, axis=mybir.AxisListType.XY)
                qp = sb.tile([128, 128], BF16, tag=f"qp{j}")
                nc.any.memset(qp[:], 0.0)
                nc.scalar.mul(qp[0:64, 0:64], qsum[0:64, :], SCALE / 9.0)
                nc.scalar.mul(qp[64:128, 64:128], qsum[64:128, :], SCALE / 9.0)
                QpBD.append(qp)

            # ---- Phase 6: stage-2 scores + exp + normalize (emitted
            # per-octet; octet 0 is interleaved into phase 5 so the softmax
            # epilogue engines start early) ----
            def s2_chain(oc):
                for t, (toff, rows) in enumerate(TOK):
                    lo = toff if rows == 128 else 512
                    ps = ps_big.tile([128, 512], F32, tag="pbig")
                    for pz in range(4):
                        pp = 4 * oc + pz
                        nc.tensor.matmul(
                            ps[:, 128 * pz : 128 * (pz + 1)],
                            qkT[pp][:, lo : lo + 128],
                            QpBD[pp][:],
                            start=True,
                            stop=True,
                        )
                    s2e = sb.tile([128, 512], F32, tag="s2e", bufs=4)
                    nc.scalar.activation(s2e[0:rows, :], ps[0:rows, :], EXP)
                    s2s = sb.tile([128, 8], F32, tag="s2s", bufs=4)
                    nc.vector.reduce_sum(
                        s2s[0:rows, :],
                        s2e[0:rows, :].rearrange("p (h q) -> p h q", q=64),
                        axis=mybir.AxisListType.X,
                    )
                    r2 = sb.tile([128, 8], F32, tag="r2", bufs=4)
                    nc.vector.reciprocal(r2[0:rows, :], s2s[0:rows, :])
                    a2 = sb.tile([128, 512], BF16, tag=f"a2n{t}_{oc}", bufs=1)
                    for pz in range(4):
                        eng = nc.vector if pz == 0 else nc.gpsimd
                        zs = slice(128 * pz, 128 * (pz + 1))
                        eng.tensor_tensor(
                            a2[0:rows, zs].rearrange("p (h q) -> p h q", q=64),
                            s2e[0:rows, zs].rearrange("p (h q) -> p h q", q=64),
                            r2[0:rows, 2 * pz : 2 * pz + 2]
                            .unsqueeze(2)
                            .broadcast_to((rows, 2, 64)),
                            op=mybir.AluOpType.mult,
                        )
                    A2n[t][oc] = a2


            # ---- Phase 5: stage-1 attention per head-pair (1-pair skew so
            # the PE never waits on the exp of the pair it just scored) ----
            QdBD = []
            s1_state = {}

            def s1_score(pp):
                a1 = sb.tile([128, 640], BF16, tag="a1", bufs=3)
                esum = sb.tile([128, 2], F32, tag="esum", bufs=3)
                for ci, (n0, nw) in enumerate(NF):
                    ps = ps_big.tile([128, nw], F32, tag="pbig")
                    nc.tensor.matmul(
                        ps[:],
                        QpBD[pp][:],
                        qkT[8 + pp][:, n0 : n0 + nw],
                        start=True,
                        stop=True,
                    )
                    nc.scalar.activation(
                        a1[:, n0 : n0 + nw],
                        ps[:],
                        EXP,
                        accum_out=esum[:, ci : ci + 1],
                    )
                nc.any.memset(a1[:, 577:640], 0.0)
                ssum = sb.tile([128, 1], F32, tag="ssum", bufs=3)
                nc.vector.tensor_add(ssum[:], esum[:, 0:1], esum[:, 1:2])
                r1 = sb.tile([128, 1], F32, tag="r1", bufs=4)
                nc.vector.reciprocal(r1[:], ssum[:])
                s1_state[pp] = (a1, r1)

            a1t_state = {}

            def s1_transpose(pp):
                a1, r1 = s1_state.pop(pp)
                # transpose A1 -> [n, 2q], full 128-wide tail (zero padded)
                pa = ps_big.tile([128, 512], BF16, tag="pbig")
                for t in range(4):
                    nc.tensor.transpose(
                        pa[:, 128 * t : 128 * (t + 1)],
                        a1[:, 128 * t : 128 * (t + 1)],
                        ident[:],
                    )
                pb = ps_small.tile([128, 128], BF16, tag="psmall")
                nc.tensor.transpose(pb[:], a1[:, 512:640], ident[:])
                a1t = sb.tile([128, 640], BF16, tag="a1t", bufs=3)
                nc.any.tensor_copy(a1t[:, 0:512], pa[:])
                nc.any.tensor_copy(a1t[:, 512:640], pb[:])
                a1t_state[pp] = (a1t, r1)

            def s1_qd(pp):
                a1t, r1 = a1t_state.pop(pp)
                # Qd pair product [2q, 2dv]; keep diagonal blocks, scaled by r1
                qd_ps = ps_small.tile([128, 128], F32, tag="psmall")
                for t in range(5):
                    nc.tensor.matmul(
                        qd_ps[:],
                        a1t[:, 128 * t : 128 * (t + 1)],
                        V[t][:, 128 * pp : 128 * (pp + 1)],
                        start=(t == 0),
                        stop=(t == 4),
                    )
                qd = sb.tile([128, 128], BF16, tag=f"qd{pp}")
                nc.any.memset(qd[:], 0.0)
                nc.vector.tensor_scalar_mul(
                    qd[0:64, 0:64], qd_ps[0:64, 0:64], r1[0:64, 0:1]
                )
                nc.vector.tensor_scalar_mul(
                    qd[64:128, 64:128], qd_ps[64:128, 64:128], r1[64:128, 0:1]
                )
                QdBD.append(qd)

            A2n = [[None, None] for _ in range(5)]
            for pp in range(8):
                s1_score(pp)
                if pp > 0:
                    s1_transpose(pp - 1)
                if pp > 1:
                    s1_qd(pp - 2)
                if pp == 2:
                    s2_chain(0)
                if pp == 6:
                    s2_chain(1)
            s1_transpose(7)
            s1_qd(6)
            s1_qd(7)


            # ---- Phase 7: transpose A2, outT = QdBD.T @ A2T -> [c, n],
            # with outT matmuls lagging one pair behind the transposes ----
            outT = []
            a2t_state = {}

            def a2_transpose(pp):
                oc, sl = pp // 4, 128 * (pp % 4)
                pa = ps_big.tile([128, 512], BF16, tag="pbig")
                for t in range(4):
                    nc.tensor.transpose(
                        pa[:, 128 * t : 128 * (t + 1)],
                        A2n[t][oc][:, sl : sl + 128],
                        ident[:],
                    )
                pb = ps_small.tile([128, 128], BF16, tag="psmall")
                nc.tensor.transpose(
                    pb[:, 0:65], A2n[4][oc][0:65, sl : sl + 128], ident[0:65, 0:65]
                )
                a2t = sb.tile([128, 640], BF16, tag="a2t", bufs=4)
                nc.any.tensor_copy(a2t[:, 0:512], pa[:])
                nc.any.tensor_copy(a2t[:, 512:577], pb[:, 0:65])
                nc.any.memset(a2t[:, 577:640], 0.0)
                a2t_state[pp] = a2t

            def out_mm(pp):
                a2t = a2t_state.pop(pp)
                oa = ps_big.tile([128, 512], F32, tag="pbig")
                ob = ps_big.tile([128, 128], F32, tag="pbig")
                nc.tensor.matmul(
                    oa[:], QdBD[pp][:], a2t[:, 0:512], start=True, stop=True
                )
                nc.tensor.matmul(
                    ob[:], QdBD[pp][:], a2t[:, 512:640], start=True, stop=True
                )
                ot = sb.tile([128, 640], BF16, tag=f"ot{pp}", bufs=1)
                nc.any.tensor_copy(ot[:, 0:512], oa[:])
                nc.any.tensor_copy(ot[:, 512:640], ob[:])
                outT.append(ot)

            a2_transpose(0)
            a2_transpose(1)
            for pp in range(2, 8):
                a2_transpose(pp)
                out_mm(pp - 2)
            out_mm(6)
            out_mm(7)

            # ---- Phase 8: output projection + bias, then to DRAM ----
            build_bias()
            for t, (toff, rows) in enumerate(TOK):
                lo = toff if rows == 128 else 512
                for half in range(2):
                    cs = slice(512 * half, 512 * (half + 1))
                    ps = ps_big.tile([128, 512], F32, tag="pbig")
                    for cc in range(8):
                        nc.tensor.matmul(
                            ps[:],
                            outT[cc][:, lo : lo + 128],
                            wp[cc][:, cs],
                            start=(cc == 0),
                            stop=(cc == 7),
                        )
                    y = sb.tile([128, 512], F32, tag="y", bufs=3)
                    nc.vector.tensor_add(y[0:rows, :], ps[0:rows, :], bias[0:rows, cs])
                    nc.sync.dma_start(out_d[b, toff : toff + rows, cs], y[0:rows, :])

        for p in (ps_small, ps_big, sb, w_pool, const_pool):
            p.release()

    nc.compile()
    return nc


_NC_CACHE = {}


def _get_nc(nb: int = NB):
    if nb not in _NC_CACHE:
        _NC_CACHE[nb] = build_program(nb)
    return _NC_CACHE[nb]


def kernel(X, W_qkv, W_proj, b_proj, layer_idx=None):
    assert X.shape == (B, N, C)
    nc = _get_nc(NB)
    xt = np.zeros((B, C, 640), dtype=np.float32)
    xt[:, :, :N] = np.asarray(X, dtype=np.float32).transpose(0, 2, 1)
    xb = xt.astype(ml_dtypes.bfloat16)
    wqkvt = np.ascontiguousarray(np.asarray(W_qkv, dtype=np.float32).T).astype(
        ml_dtypes.bfloat16
    )
    wprojt = np.ascontiguousarray(np.asarray(W_proj, dtype=np.float32).T).astype(
        ml_dtypes.bfloat16
    )
    wbias = np.asarray(b_proj, dtype=np.float32).reshape(1, C).astype(
        ml_dtypes.bfloat16
    )
    in_maps = [
        {
            "xt": xb[NB * i : NB * (i + 1)],
            "wqkvt": wqkvt,
            "wprojt": wprojt,
            "wbias": wbias,
        }
        for i in range(N_CORES)
    ]
    res = run_bass_kernel_spmd(nc, in_maps, core_ids=list(range(N_CORES)))
    out = np.concatenate([res.results[i]["out"] for i in range(N_CORES)], axis=0)
    return out.astype(np.float32)


if __name__ == "__main__":
    rng = np.random.default_rng(0)
    X = rng.standard_normal((B, N, C), dtype=np.float32)
    W_qkv = rng.standard_normal((3 * C, C), dtype=np.float32) * C**-0.5
    W_proj = rng.standard_normal((C, C), dtype=np.float32) * C**-0.5
    b_proj = np.zeros(C, dtype=np.float32)
    out = kernel(X, W_qkv, W_proj, b_proj, 1)
    print(out.shape, out.dtype)
